# revision 25
# baseline (speedup 1.0000x reference)
"""Trainium2 Bass kernel for nn_FFTChainMatrix (block-circulant matmul via 64-pt rFFT).

y = x @ W.T with W 4096x4096 block-circulant, computed in the FFT domain as a
3-matmul pipeline (rfft -> per-freq contraction -> irfft) with two SBUF
per-frequency shuffle DMA stages between the matmuls.

Final design (~104 us vs ~164 us baseline):
  - x/y live in DRAM in the exact SBUF layout (host pre/post-transposes), so
    loads/stores are fully-contiguous DMAs (~350 GB/s) instead of 1 KiB-run
    gathers (~17 GB/s effective).
  - spread-port shuffles: each shuffle DMA's 4 source partitions are
    stride-4 (4 distinct SBUF AXI ports, via the cport column permutation of
    A2/G), and the issue order interleaves 4 disjoint port sets per window.
  - all shuffle DMAs issue from gpsimd (SWDGE) with single_packet=True: its
    deep descriptor rings keep many DMAs in flight; adding sync/scalar
    issue queues was measured strictly slower (shallow HWDGE rings stall
    and fragment the SDMA packet round-robin).
  - intermediate column layouts are pos-major (shuffle-issue order) so
    position pairs share [128,1024] PSUM tiles -> half the PSUM->SBUF
    copyback ops; copybacks greedy-balanced between DVE and ACT.
  - A2/B2 load first (64 KiB) so S1 starts on the first x chunk; S2/S3
    matmuls chase the shuffle arrivals pos-by-pos; stores chase S3 per
    4-position chunk; warm matmuls bridge the shuffle barriers to keep the
    PE p-state hot.

Rejected on measurement: DVE InstStreamTranspose shuffle (strided stream ->
4.6 ns/elem, 76 us/stage), PE-transpose shuffle (the 512-token rider makes
element transposes quadratic in ops), token-halved A/B pipelining (512 B
descriptors drop the shuffle rate more than the overlap gains), 4-dim or
mid-AP-partition DMA patterns (illegal: max 3 AP dims, partition must be
the leading dim).

Per-core data layout (T=512 tokens/core, f16):
  x_sb [128=(64j+d), ib*512 + t]                (in-block i = 2*ib + j)
  S1: out = A2.T @ x   -> X1 [128=cport(fp, 2z+j), (ib,t)]
  shuf fp: X2[(2z+j)*32+ib, pos(fp)*512+t] = X1[cport(fp,2z+j), ib*512+t]
           (cport(b,q) = 16*(b//4) + 4q + b%4)
  S2: out = G[fp].T @ X2 -> Y2 [128=cport(ob, 2zo+jo), (pos,t)]
  unshuf ob: Y3[(2zo+jo)*32+POS[fp], pos(ob)*512+t] = Y2[cport(ob,..), ..]
  S3: out = B2.T @ Y3 -> ys [128=(64jo+d), (pos,t)]   (out o = 2*ob + jo)

Sharding: data-parallel over tokens, 4096 tokens -> 8 cores x 512.
"""

from contextlib import ExitStack

import numpy as np

BLK = 64
NB = 64           # circulant blocks per side
# port-interleaved issue order: consecutive shuffle DMAs read disjoint
# SBUF-AXI-port sets; intermediate column layouts are "pos-major" (ordered
# by this sequence) so adjacent positions share PSUM tiles and stores.
PORT_ORDER = [b + 16 * c + 4 * a
              for b in range(4) for c in range(2) for a in (0, 1)]
PORT_ORDER += [8 + v for v in PORT_ORDER]
POS = [0] * 32
for _i, _v in enumerate(PORT_ORDER):
    POS[_v] = _i
T = 512           # tokens per core
NCORES = 8
FEAT = 4096
NC_COLS = 32 * T  # 16384 sbuf cols per full tile


# ---------------------------------------------------------------- host math
def _build_matrices(circulant_params, channel_weights):
    """A2 [128,128], G [32,128,128], B2 [128,128] (float64 math)."""
    c_w = np.einsum(
        "m,moid->oid",
        np.asarray(channel_weights, np.float64),
        np.asarray(circulant_params, np.float64),
    )
    Chat = np.fft.rfft(c_w, axis=-1)
    Wr, Wi = Chat.real, Chat.imag

    r = np.arange(BLK)
    A64 = np.zeros((BLK, BLK))
    A64[0, :] = 1.0
    A64[1, :] = (-1.0) ** r
    B64 = np.zeros((BLK, BLK))
    B64[:, 0] = 1.0 / BLK
    B64[:, 1] = ((-1.0) ** r) / BLK
    for p in range(1, 32):
        cc = np.cos(2 * np.pi * p * r / BLK)
        ss = np.sin(2 * np.pi * p * r / BLK)
        A64[2 * p, :] = cc
        A64[2 * p + 1, :] = -ss
        B64[:, 2 * p] = 2.0 * cc / BLK
        B64[:, 2 * p + 1] = -2.0 * ss / BLK

    # Spread-port column index: the 4 source partitions of one shuffle DMA
    # land on 4 distinct SBUF AXI ports (stride-4 partitions) instead of 1.
    def cport(blk4, q):
        return 16 * (blk4 // 4) + 4 * q + (blk4 % 4)

    # A2[(64j + d), cport(fp, 2z+j)] = A64[2fp+z, d]
    # B2[(2zo + jo)*32 + fp, (64jo + d)] = B64[d, 2fp + zo]
    A2 = np.zeros((128, 128))
    B2 = np.zeros((128, 128))
    for j in range(2):
        for z in range(2):
            for fp in range(32):
                A2[64 * j: 64 * j + 64, cport(fp, 2 * z + j)] = A64[2 * fp + z, :]
    for zo in range(2):
        for jo in range(2):
            for fp in range(32):
                B2[(2 * zo + jo) * 32 + POS[fp], 64 * jo: 64 * jo + 64] = \
                    B64[:, 2 * fp + zo]

    # G[fp][((2z + ji)*32 + ib), cport(ob, 2zo + jo)]   (i = 2ib+ji, o = 2ob+jo)
    # z/zo: 0 = Re(F_fp), 1 = Im(F_fp)  (for fp=0: 0 = F_0, 1 = F_32, both real)
    i = np.arange(NB)
    rows = (2 * np.arange(2)[None, :] + (i % 2)[:, None]) * 32 + (i // 2)[:, None]
    G = np.zeros((32, 128, 128))
    blk = np.zeros((NB, 2, NB, 2))
    for fp in range(32):
        blk[:] = 0.0
        if fp == 0:
            blk[:, 0, :, 0] = Wr[:, :, 0].T
            blk[:, 1, :, 1] = Wr[:, :, 32].T
        else:
            blk[:, 0, :, 0] = Wr[:, :, fp].T
            blk[:, 1, :, 0] = -Wi[:, :, fp].T
            blk[:, 0, :, 1] = Wi[:, :, fp].T
            blk[:, 1, :, 1] = Wr[:, :, fp].T
        cols = (16 * ((i // 2) // 4) + ((i // 2) % 4))[:, None] + \
            4 * (2 * np.arange(2)[None, :] + (i % 2)[:, None])
        G[fp][rows[:, :, None, None], cols[None, None, :, :]] = blk
    return A2, G, B2


# ---------------------------------------------------------------- bass trace
def _trace_nc():
    import concourse.bass as bass  # noqa: F401
    import concourse.mybir as mybir
    import concourse.tile as tile
    from concourse import bacc

    f32 = mybir.dt.float32
    f16 = mybir.dt.float16

    nc = bacc.Bacc("TRN2", target_bir_lowering=False, debug=False,
                   num_devices=NCORES)
    x_h = nc.dram_tensor("x_shard", [128, NC_COLS], f16,
                         kind="ExternalInput").ap()
    w_h = nc.dram_tensor("w_mats", [128, 4352], f16, kind="ExternalInput").ap()
    y_h = nc.dram_tensor("y_shard", [128, NC_COLS], f16,
                         kind="ExternalOutput").ap()

    eng_cost = [0.0, 0.0]  # [vector, scalar] modeled busy ns
    dma_ix = [0]

    with tile.TileContext(nc) as tc, ExitStack() as ctx:
        wpool = ctx.enter_context(tc.tile_pool(name="weights", bufs=1))
        xpool = ctx.enter_context(tc.tile_pool(name="xin", bufs=8))
        x1pool = ctx.enter_context(tc.tile_pool(name="x1u", bufs=1))
        x2pool = ctx.enter_context(tc.tile_pool(name="x2sb", bufs=1))
        y2pool = ctx.enter_context(tc.tile_pool(name="y2u", bufs=1))
        y3pool = ctx.enter_context(tc.tile_pool(name="y3sb", bufs=1))
        ypool = ctx.enter_context(tc.tile_pool(name="yout", bufs=8))
        wmpool = ctx.enter_context(tc.tile_pool(name="warm", bufs=1))
        mmps = ctx.enter_context(tc.tile_pool(name="mmps", bufs=4, space="PSUM"))

        # PSUM->SBUF copyback: only DVE/Act can read PSUM.  Greedy-balance
        # by modeled per-op cost.
        def cb(dst, src, n=1024):
            cost_v = n * 1.04 + 125.0
            cost_s = n / 1.2 + 143.0
            if eng_cost[0] + cost_v <= eng_cost[1] + cost_s:
                eng_cost[0] += cost_v
                nc.vector.tensor_copy(dst, src)
            else:
                eng_cost[1] += cost_s
                nc.scalar.copy(dst, src)

        def dma(dst, src):
            eng = (nc.sync, nc.gpsimd)[dma_ix[0] % 2]
            dma_ix[0] += 1
            eng.dma_start(dst, src)

        def dma_g(dst, src):
            # shuffle stages: gpsimd-dominant -- SWDGE's deep rings keep
            # many DMAs in flight; sync adds every 8th without becoming
            # the phase tail.
            k = dma_ix[0]
            dma_ix[0] += 1
            eng = nc.sync if k % 8 == 7 else nc.gpsimd
            eng.dma_start(dst, src, single_packet=True)

        # ---- weights: A2/B2 first (64 KiB, unblocks S1), then G; x in 8
        # contiguous 512 KiB chunks chased by S1.
        wt = wpool.tile([128, 4352], f16)
        nc.sync.dma_start(wt[:, 4096:4352], w_h[:, 4096:4352])
        a2 = wt[:, 4096:4224]
        b2 = wt[:, 4224:4352]

        xsb = [xpool.tile([128, 4 * T], f16, tag="xsb", name=f"xsb{h}")
               for h in range(8)]
        for h in range(8):
            dma(xsb[h][:], x_h[:, h * 4 * T:(h + 1) * 4 * T])
        nc.gpsimd.dma_start(wt[:, 0:4096], w_h[:, 0:4096])

        x1u = x1pool.tile([128, NC_COLS], f16)
        x2sb = x2pool.tile([128, NC_COLS], f16)
        y2u = y2pool.tile([128, NC_COLS], f16)
        y3sb = y3pool.tile([128, NC_COLS], f16)
        ys = [ypool.tile([128, 4 * T], f16, tag="ys", name=f"ys{h}")
              for h in range(8)]

        # ---- PE warm stream: ramp the HAM p-state while the x loads land,
        # and keep it hot across the shuffle barriers.
        warm = wmpool.tile([128, 512], f16)
        nc.vector.memset(warm[:], 0.0)

        def warm_mm(n):
            for _ in range(n):
                ps = mmps.tile([128, 1024], f32, tag="mm")
                nc.tensor.matmul(ps[:, 0:512], warm[:, 0:128], warm[:],
                                 start=True, stop=True)

        warm_mm(10)

        # ---- S1 (rfft): stationary A2, moving x[ib]; chases the loads
        for ib2 in range(16):
            ps = mmps.tile([128, 1024], f32, tag="mm")
            for k in range(2):
                ib = 2 * ib2 + k
                nc.tensor.matmul(ps[:, k * T:(k + 1) * T], a2,
                                 xsb[ib // 4][:, (ib % 4) * T:(ib % 4 + 1) * T],
                                 start=True, stop=True)
            cb(x1u[:, ib2 * 1024:(ib2 + 1) * 1024], ps[:])
        warm_mm(20)

        # ---- shuffle: X1 partitions cport(fp, q) across all ib -> X2 pos-major
        for pos, fp in enumerate(PORT_ORDER):
            p0 = 16 * (fp // 4) + (fp % 4)
            src = x1u[p0:p0 + 13:4, :].rearrange("p (ib t) -> p ib t", t=T)
            dma_g(x2sb[:, pos * T:(pos + 1) * T], src)

        # ---- S2: per-freq-pair complex contraction; chases the shuffle
        for i2 in range(16):
            ps = mmps.tile([128, 1024], f32, tag="mm")
            for k in range(2):
                pos = 2 * i2 + k
                fp = PORT_ORDER[pos]
                nc.tensor.matmul(ps[:, k * T:(k + 1) * T],
                                 wt[:, fp * 128:(fp + 1) * 128],
                                 x2sb[:, pos * T:(pos + 1) * T],
                                 start=True, stop=True)
            cb(y2u[:, i2 * 1024:(i2 + 1) * 1024], ps[:])
        warm_mm(20)

        # ---- unshuffle: Y2 partitions cport(ob, w) -> Y3 pos-major
        for pos, ob in enumerate(PORT_ORDER):
            p0 = 16 * (ob // 4) + (ob % 4)
            src = y2u[p0:p0 + 13:4, :].rearrange("p (f t) -> p f t", t=T)
            dma_g(y3sb[:, pos * T:(pos + 1) * T], src)

        # ---- S3 (irfft) + stores chase in 8 chunks of 4 positions
        for i2 in range(16):
            ps = mmps.tile([128, 1024], f32, tag="mm")
            for k in range(2):
                pos = 2 * i2 + k
                nc.tensor.matmul(ps[:, k * T:(k + 1) * T], b2,
                                 y3sb[:, pos * T:(pos + 1) * T],
                                 start=True, stop=True)
            cb(ys[i2 // 2][:, (i2 % 2) * 1024:(i2 % 2 + 1) * 1024], ps[:])
            if i2 % 2 == 1:
                h = i2 // 2
                nc.sync.dma_start(y_h[:, h * 4 * T:(h + 1) * 4 * T], ys[h][:])

    nc.compile()
    return nc


_CACHE = {}


def make_in_maps(x, circulant_params, channel_weights):
    xf = np.asarray(x, np.float32).reshape(-1, FEAT)
    assert xf.shape[0] == NCORES * T, f"unexpected token count {xf.shape}"
    A2, G, B2 = _build_matrices(circulant_params, channel_weights)
    w = np.zeros((128, 4352), np.float16)
    w[:, 0:4096] = G.transpose(1, 0, 2).reshape(128, 4096).astype(np.float16)
    w[:, 4096:4224] = A2.astype(np.float16)
    w[:, 4224:4352] = B2.astype(np.float16)
    # x_dev[c][64j+d, ib*512 + t] = x[c*512 + t, 64*(2ib+j) + d]
    xd = xf.astype(np.float16).reshape(NCORES, T, 32, 2, 64)
    xd = np.ascontiguousarray(xd.transpose(0, 3, 4, 2, 1)).reshape(
        NCORES, 128, NC_COLS)
    return [
        {"x_shard": xd[c], "w_mats": w}
        for c in range(NCORES)
    ]


def kernel(x, circulant_params, channel_weights):
    from concourse.bass_utils import run_bass_kernel_spmd

    x = np.asarray(x, np.float32)
    orig_shape = x.shape

    if "nc" not in _CACHE:
        _CACHE["nc"] = _trace_nc()
    nc = _CACHE["nc"]

    in_maps = make_in_maps(x, circulant_params, channel_weights)
    res = run_bass_kernel_spmd(nc, in_maps, core_ids=list(range(NCORES)))
    # y_dev[c][64jo+d, ob*512 + t] = y[c*512 + t, 128*ob + 64*jo + d]
    yd = np.stack([res.results[c]["y_shard"] for c in range(NCORES)])
    yd = yd.reshape(NCORES, 2, 64, 32, T)[:, :, :, POS, :]
    yd = yd.transpose(0, 4, 3, 1, 2)
    return np.ascontiguousarray(yd).reshape(orig_shape).astype(np.float32)


# revision 26
# speedup vs baseline: 1.0859x; 1.0859x over previous
"""Trainium2 Bass kernel for nn_FFTChainMatrix (block-circulant matmul via 64-pt rFFT).

y = x @ W.T with W 4096x4096 block-circulant, computed in the FFT domain as a
3-matmul pipeline (rfft -> per-freq contraction -> irfft) with two SBUF
per-frequency shuffle DMA stages between the matmuls.

Final design (~104 us vs ~164 us baseline):
  - x/y live in DRAM in the exact SBUF layout (host pre/post-transposes), so
    loads/stores are fully-contiguous DMAs (~350 GB/s) instead of 1 KiB-run
    gathers (~17 GB/s effective).
  - spread-port shuffles: each shuffle DMA's 4 source partitions are
    stride-4 (4 distinct SBUF AXI ports, via the cport column permutation of
    A2/G), and the issue order interleaves 4 disjoint port sets per window.
  - all shuffle DMAs issue from gpsimd (SWDGE) with single_packet=True: its
    deep descriptor rings keep many DMAs in flight; adding sync/scalar
    issue queues was measured strictly slower (shallow HWDGE rings stall
    and fragment the SDMA packet round-robin).
  - intermediate column layouts are pos-major (shuffle-issue order) so
    position pairs share [128,1024] PSUM tiles -> half the PSUM->SBUF
    copyback ops; copybacks greedy-balanced between DVE and ACT.
  - A2/B2 load first (64 KiB) so S1 starts on the first x chunk; S2/S3
    matmuls chase the shuffle arrivals pos-by-pos; stores chase S3 per
    4-position chunk; warm matmuls bridge the shuffle barriers to keep the
    PE p-state hot.

Rejected on measurement: DVE InstStreamTranspose shuffle (strided stream ->
4.6 ns/elem, 76 us/stage), PE-transpose shuffle (the 512-token rider makes
element transposes quadratic in ops), token-halved A/B pipelining (512 B
descriptors drop the shuffle rate more than the overlap gains), 4-dim or
mid-AP-partition DMA patterns (illegal: max 3 AP dims, partition must be
the leading dim).

Per-core data layout (T=512 tokens/core, f16):
  x_sb [128=(64j+d), ib*512 + t]                (in-block i = 2*ib + j)
  S1: out = A2.T @ x   -> X1 [128=cport(fp, 2z+j), (ib,t)]
  shuf fp: X2[(2z+j)*32+ib, pos(fp)*512+t] = X1[cport(fp,2z+j), ib*512+t]
           (cport(b,q) = 16*(b//4) + 4q + b%4)
  S2: out = G[fp].T @ X2 -> Y2 [128=cport(ob, 2zo+jo), (pos,t)]
  unshuf ob: Y3[(2zo+jo)*32+POS[fp], pos(ob)*512+t] = Y2[cport(ob,..), ..]
  S3: out = B2.T @ Y3 -> ys [128=(64jo+d), (pos,t)]   (out o = 2*ob + jo)

Sharding: data-parallel over tokens, 4096 tokens -> 8 cores x 512.
"""

from contextlib import ExitStack

import numpy as np

BLK = 64
NB = 64           # circulant blocks per side
# port-interleaved issue order: consecutive shuffle DMAs read disjoint
# SBUF-AXI-port sets; intermediate column layouts are "pos-major" (ordered
# by this sequence) so adjacent positions share PSUM tiles and stores.
PORT_ORDER = [b + 16 * c + 4 * a
              for b in range(4) for c in range(2) for a in (0, 1)]
PORT_ORDER += [8 + v for v in PORT_ORDER]
POS = [0] * 32
for _i, _v in enumerate(PORT_ORDER):
    POS[_v] = _i
T = 512           # tokens per core
NCORES = 8
FEAT = 4096
NC_COLS = 32 * T  # 16384 sbuf cols per full tile


# ---------------------------------------------------------------- host math
def _build_matrices(circulant_params, channel_weights):
    """A2 [128,128], G [32,128,128], B2 [128,128] (float64 math)."""
    c_w = np.einsum(
        "m,moid->oid",
        np.asarray(channel_weights, np.float64),
        np.asarray(circulant_params, np.float64),
    )
    Chat = np.fft.rfft(c_w, axis=-1)
    Wr, Wi = Chat.real, Chat.imag

    r = np.arange(BLK)
    A64 = np.zeros((BLK, BLK))
    A64[0, :] = 1.0
    A64[1, :] = (-1.0) ** r
    B64 = np.zeros((BLK, BLK))
    B64[:, 0] = 1.0 / BLK
    B64[:, 1] = ((-1.0) ** r) / BLK
    for p in range(1, 32):
        cc = np.cos(2 * np.pi * p * r / BLK)
        ss = np.sin(2 * np.pi * p * r / BLK)
        A64[2 * p, :] = cc
        A64[2 * p + 1, :] = -ss
        B64[:, 2 * p] = 2.0 * cc / BLK
        B64[:, 2 * p + 1] = -2.0 * ss / BLK

    # Spread-port column index: the 4 source partitions of one shuffle DMA
    # land on 4 distinct SBUF AXI ports (stride-4 partitions) instead of 1.
    def cport(blk4, q):
        return 16 * (blk4 // 4) + 4 * q + (blk4 % 4)

    # A2[(64j + d), cport(fp, 2z+j)] = A64[2fp+z, d]
    # B2[(2zo + jo)*32 + fp, (64jo + d)] = B64[d, 2fp + zo]
    A2 = np.zeros((128, 128))
    B2 = np.zeros((128, 128))
    for j in range(2):
        for z in range(2):
            for fp in range(32):
                A2[64 * j: 64 * j + 64, cport(fp, 2 * z + j)] = A64[2 * fp + z, :]
    for zo in range(2):
        for jo in range(2):
            for fp in range(32):
                B2[(2 * zo + jo) * 32 + POS[fp], 64 * jo: 64 * jo + 64] = \
                    B64[:, 2 * fp + zo]

    # G[fp][((2z + ji)*32 + ib), cport(ob, 2zo + jo)]   (i = 2ib+ji, o = 2ob+jo)
    # z/zo: 0 = Re(F_fp), 1 = Im(F_fp)  (for fp=0: 0 = F_0, 1 = F_32, both real)
    i = np.arange(NB)
    rows = (2 * np.arange(2)[None, :] + (i % 2)[:, None]) * 32 + (i // 2)[:, None]
    G = np.zeros((32, 128, 128))
    blk = np.zeros((NB, 2, NB, 2))
    for fp in range(32):
        blk[:] = 0.0
        if fp == 0:
            blk[:, 0, :, 0] = Wr[:, :, 0].T
            blk[:, 1, :, 1] = Wr[:, :, 32].T
        else:
            blk[:, 0, :, 0] = Wr[:, :, fp].T
            blk[:, 1, :, 0] = -Wi[:, :, fp].T
            blk[:, 0, :, 1] = Wi[:, :, fp].T
            blk[:, 1, :, 1] = Wr[:, :, fp].T
        cols = (16 * ((i // 2) // 4) + ((i // 2) % 4))[:, None] + \
            4 * (2 * np.arange(2)[None, :] + (i % 2)[:, None])
        G[fp][rows[:, :, None, None], cols[None, None, :, :]] = blk
    return A2, G, B2


# ---------------------------------------------------------------- bass trace
def _trace_nc():
    import concourse.bass as bass  # noqa: F401
    import concourse.mybir as mybir
    import concourse.tile as tile
    from concourse import bacc

    f32 = mybir.dt.float32
    f16 = mybir.dt.float16

    nc = bacc.Bacc("TRN2", target_bir_lowering=False, debug=False,
                   num_devices=NCORES)
    x_h = nc.dram_tensor("x_shard", [128, NC_COLS], f16,
                         kind="ExternalInput").ap()
    w_h = nc.dram_tensor("w_mats", [128, 4352], f16, kind="ExternalInput").ap()
    y_h = nc.dram_tensor("y_shard", [128, NC_COLS], f16,
                         kind="ExternalOutput").ap()

    eng_cost = [0.0, 0.0]  # [vector, scalar] modeled busy ns
    dma_ix = [0]

    with tile.TileContext(nc) as tc, ExitStack() as ctx:
        wpool = ctx.enter_context(tc.tile_pool(name="weights", bufs=1))
        xpool = ctx.enter_context(tc.tile_pool(name="xin", bufs=8))
        x1pool = ctx.enter_context(tc.tile_pool(name="x1u", bufs=1))
        x2pool = ctx.enter_context(tc.tile_pool(name="x2sb", bufs=1))
        y2pool = ctx.enter_context(tc.tile_pool(name="y2u", bufs=1))
        y3pool = ctx.enter_context(tc.tile_pool(name="y3sb", bufs=1))
        ypool = ctx.enter_context(tc.tile_pool(name="yout", bufs=8))
        wmpool = ctx.enter_context(tc.tile_pool(name="warm", bufs=1))
        mmps = ctx.enter_context(tc.tile_pool(name="mmps", bufs=4, space="PSUM"))

        # PSUM->SBUF copyback: only DVE/Act can read PSUM.  Greedy-balance
        # by modeled per-op cost.
        def cb(dst, src, n=1024):
            cost_v = n * 1.04 + 125.0
            cost_s = n / 1.2 + 143.0
            if eng_cost[0] + cost_v <= eng_cost[1] + cost_s:
                eng_cost[0] += cost_v
                nc.vector.tensor_copy(dst, src)
            else:
                eng_cost[1] += cost_s
                nc.scalar.copy(dst, src)

        def dma(dst, src):
            eng = (nc.sync, nc.gpsimd)[dma_ix[0] % 2]
            dma_ix[0] += 1
            eng.dma_start(dst, src)

        def dma_g(dst, src):
            # shuffle stages: all on gpsimd (SWDGE) -- its deep descriptor
            # rings keep many DMAs in flight; any sync/scalar participation
            # measured strictly slower.
            dma_ix[0] += 1
            nc.gpsimd.dma_start(dst, src, single_packet=True)

        # ---- weights: A2/B2 first (64 KiB, unblocks S1), then G; x in 8
        # contiguous 512 KiB chunks chased by S1.
        wt = wpool.tile([128, 4352], f16)
        nc.sync.dma_start(wt[:, 4096:4352], w_h[:, 4096:4352])
        a2 = wt[:, 4096:4224]
        b2 = wt[:, 4224:4352]

        xsb = [xpool.tile([128, 4 * T], f16, tag="xsb", name=f"xsb{h}")
               for h in range(8)]
        for h in range(8):
            dma(xsb[h][:], x_h[:, h * 4 * T:(h + 1) * 4 * T])
        nc.gpsimd.dma_start(wt[:, 0:4096], w_h[:, 0:4096])

        x1u = x1pool.tile([128, NC_COLS], f16)
        x2sb = x2pool.tile([128, NC_COLS], f16)
        y2u = y2pool.tile([128, NC_COLS], f16)
        y3sb = y3pool.tile([128, NC_COLS], f16)
        ys = [ypool.tile([128, 4 * T], f16, tag="ys", name=f"ys{h}")
              for h in range(8)]

        # ---- PE warm stream: ramp the HAM p-state while the x loads land,
        # and keep it hot across the shuffle barriers.
        warm = wmpool.tile([128, 512], f16)
        nc.vector.memset(warm[:], 0.0)

        def warm_mm(n):
            for _ in range(n):
                ps = mmps.tile([128, 1024], f32, tag="mm")
                nc.tensor.matmul(ps[:, 0:512], warm[:, 0:128], warm[:],
                                 start=True, stop=True)

        warm_mm(10)

        # ---- S1 (rfft): stationary A2, moving x[ib]; chases the loads
        for ib2 in range(16):
            ps = mmps.tile([128, 1024], f32, tag="mm")
            for k in range(2):
                ib = 2 * ib2 + k
                nc.tensor.matmul(ps[:, k * T:(k + 1) * T], a2,
                                 xsb[ib // 4][:, (ib % 4) * T:(ib % 4 + 1) * T],
                                 start=True, stop=True)
            cb(x1u[:, ib2 * 1024:(ib2 + 1) * 1024], ps[:])
        warm_mm(20)

        # ---- shuffle: X1 partitions cport(fp, q) across all ib -> X2 pos-major
        for pos, fp in enumerate(PORT_ORDER):
            p0 = 16 * (fp // 4) + (fp % 4)
            src = x1u[p0:p0 + 13:4, :].rearrange("p (ib t) -> p ib t", t=T)
            dma_g(x2sb[:, pos * T:(pos + 1) * T], src)

        # ---- S2: per-freq-pair complex contraction; chases the shuffle
        for i2 in range(16):
            ps = mmps.tile([128, 1024], f32, tag="mm")
            for k in range(2):
                pos = 2 * i2 + k
                fp = PORT_ORDER[pos]
                nc.tensor.matmul(ps[:, k * T:(k + 1) * T],
                                 wt[:, fp * 128:(fp + 1) * 128],
                                 x2sb[:, pos * T:(pos + 1) * T],
                                 start=True, stop=True)
            cb(y2u[:, i2 * 1024:(i2 + 1) * 1024], ps[:])
        warm_mm(20)

        # ---- unshuffle: Y2 partitions cport(ob, w) -> Y3 pos-major
        for pos, ob in enumerate(PORT_ORDER):
            p0 = 16 * (ob // 4) + (ob % 4)
            src = y2u[p0:p0 + 13:4, :].rearrange("p (f t) -> p f t", t=T)
            dma_g(y3sb[:, pos * T:(pos + 1) * T], src)

        # ---- S3 (irfft) + stores chase in 8 chunks of 4 positions
        for i2 in range(16):
            ps = mmps.tile([128, 1024], f32, tag="mm")
            for k in range(2):
                pos = 2 * i2 + k
                nc.tensor.matmul(ps[:, k * T:(k + 1) * T], b2,
                                 y3sb[:, pos * T:(pos + 1) * T],
                                 start=True, stop=True)
            cb(ys[i2 // 2][:, (i2 % 2) * 1024:(i2 % 2 + 1) * 1024], ps[:])
            if i2 % 2 == 1:
                h = i2 // 2
                nc.sync.dma_start(y_h[:, h * 4 * T:(h + 1) * 4 * T], ys[h][:])

    nc.compile()
    return nc


_CACHE = {}


def make_in_maps(x, circulant_params, channel_weights):
    xf = np.asarray(x, np.float32).reshape(-1, FEAT)
    assert xf.shape[0] == NCORES * T, f"unexpected token count {xf.shape}"
    A2, G, B2 = _build_matrices(circulant_params, channel_weights)
    w = np.zeros((128, 4352), np.float16)
    w[:, 0:4096] = G.transpose(1, 0, 2).reshape(128, 4096).astype(np.float16)
    w[:, 4096:4224] = A2.astype(np.float16)
    w[:, 4224:4352] = B2.astype(np.float16)
    # x_dev[c][64j+d, ib*512 + t] = x[c*512 + t, 64*(2ib+j) + d]
    xd = xf.astype(np.float16).reshape(NCORES, T, 32, 2, 64)
    xd = np.ascontiguousarray(xd.transpose(0, 3, 4, 2, 1)).reshape(
        NCORES, 128, NC_COLS)
    return [
        {"x_shard": xd[c], "w_mats": w}
        for c in range(NCORES)
    ]


def kernel(x, circulant_params, channel_weights):
    from concourse.bass_utils import run_bass_kernel_spmd

    x = np.asarray(x, np.float32)
    orig_shape = x.shape

    if "nc" not in _CACHE:
        _CACHE["nc"] = _trace_nc()
    nc = _CACHE["nc"]

    in_maps = make_in_maps(x, circulant_params, channel_weights)
    res = run_bass_kernel_spmd(nc, in_maps, core_ids=list(range(NCORES)))
    # y_dev[c][64jo+d, ob*512 + t] = y[c*512 + t, 128*ob + 64*jo + d]
    yd = np.stack([res.results[c]["y_shard"] for c in range(NCORES)])
    yd = yd.reshape(NCORES, 2, 64, 32, T)[:, :, :, POS, :]
    yd = yd.transpose(0, 4, 3, 1, 2)
    return np.ascontiguousarray(yd).reshape(orig_shape).astype(np.float32)
